# revision 1
# baseline (speedup 1.0000x reference)
"""Decode-phase paged attention with GQA on 8 TRN2 NeuronCores — v2.

Sharding: kv-head axis across the 8 cores (1 kv head + its 4 query heads per
core); q/block_tables/context_lens replicated, metadata baked into the
per-call compiled graph (host resolves the paged gather, device streams every
gathered KV byte from HBM — the memory-bound roofline term).

v2 layout vs v1:
- Long sequences are packed CONTINUOUSLY at 32-token granularity (v1 padded
  every sequence to 128): fewer HBM bytes. Sequences start mid-chunk;
  matmuls address partition offsets via tile_position (fragments split so
  (offset, len) hits legal 32-aligned placements).
- K, V and a ones-column are interleaved per 128-token chunk in ONE dram
  tensor ([d,128]K | [128,128]V | ones), so each batch is a single ~2 MB DMA
  on the sync queue — no second queue serialized behind compute.
- exp() is emitted per 16-chunk group, coalescing full chunks into one ACT
  instruction; boundary chunks get a host-built -60 bias column (masks both
  ctx tail and 32-pad rows).
- The softmax divide moved to the host: the device ships numerator and
  denominator ([G, 129] per seq, PSUM->SBUF drained on DVE), which takes the
  reciprocal+normalize chain off the critical tail and keeps ACT exp-only.
- Batch sizes taper (32,... then 8-chunk tail batches) so the compute tail
  after the last DMA byte is tiny.
- Sequences are packed shortest-first (the tail after the last DMA is one
  long sequence's final chunks), and a chunk never holds more than 3 matmul
  fragments — 4-fragment chunks (four 32-row tile_position'd matmuls) hard-
  fault the device; starts bump to the next chunk boundary when needed.
"""

import sys

if "/opt/trn_rl_repo" not in sys.path:
    sys.path.insert(0, "/opt/trn_rl_repo")

import numpy as np
import ml_dtypes

B = 64
H = 32
KVH = 8
G = H // KVH  # 4
D = 128
BS = 16  # tokens per cache block
NB = 8192  # blocks in cache
CH = 128  # tokens per chunk (compute tile)
CW = 2 * D + 1  # kv stream cols per chunk: K(128) | V(128) | ones(1)
E = D + 1  # per-seq output cols: numerator(128) | denominator(1)
ALIGN = 32  # long-seq token padding granularity
SHORT_MAX = 384  # ctx at/below this pads to full chunks (fragment-run guard)
NCHG = 16  # chunks per st/pt group (one exp batch)
BATCH_CHUNKS = 32  # chunks per KV buffer
SCALE = 0.08838834764831845
NCORES = 8
NEG = -60.0

BF16 = ml_dtypes.bfloat16


def _split_frag(r0, lp):
    """Split a (row0, len) piece so each part is a legal tile placement.

    Legal (pos, size): pos 0 any size; pos 32 size<=32; pos 64 size<=64;
    pos 96 size<=32 (round-up sizes 32/64/128 vs 32-aligned positions).
    """
    if r0 == 32 and lp > 32:
        return [(32, 32), (64, lp - 32)]
    return [(r0, lp)]


def plan_problem(block_tables, context_lens):
    bt = np.asarray(block_tables, dtype=np.int64)
    ctx_orig = np.asarray(context_lens, dtype=np.int64)

    # pack shortest-first so the post-last-DMA tail is one long sequence's
    # final chunks -> shortest possible compute/drain chain at the end
    order = np.argsort(ctx_orig, kind="stable")
    ctx = ctx_orig[order]
    bts = bt[order]

    # Short seqs pad to full 128-token chunks; long seqs pad to 32. Runs of
    # 3+ consecutive fragment-dense chunks hard-fault the device, so short
    # seqs (which would pack several 32-row fragments per chunk back-to-
    # back) get whole chunks to themselves, and long-seq boundary chunks
    # are always separated by >=2 full interior chunks. Sorted ascending,
    # the shorts come first with 128-aligned starts for free.
    short = ctx <= SHORT_MAX
    ltok = np.where(
        short,
        np.maximum(CH, ((ctx + CH - 1) // CH) * CH),
        ((ctx + ALIGN - 1) // ALIGN) * ALIGN,
    )

    # sequential packing with a fragment cap: a chunk must never hold 4+
    # matmul fragments (dense 32-row fragment runs hard-fault the device).
    # If starting a seq mid-chunk would push that chunk past 3 fragments,
    # bump the seq to the next chunk boundary (the skipped rows stay
    # uncovered: no fragment touches them and exps exclude them via rmax).
    starts = np.zeros(B, dtype=np.int64)
    frag_cnt = {}
    cur = 0
    for p in range(B):
        L = int(ltok[p])
        r = cur % CH
        if r:
            head = min(L, CH - r)
            if frag_cnt.get(cur // CH, 0) + len(_split_frag(r, head)) > 3:
                cur = (cur // CH + 1) * CH
        starts[p] = cur
        t = cur
        end = cur + L
        while t < end:
            c = t // CH
            r0 = t % CH
            lp = min(end - t, CH - r0)
            frag_cnt[c] = frag_cnt.get(c, 0) + len(_split_frag(r0, lp))
            t += lp
        cur = end
    total = cur
    nchunk = (total + CH - 1) // CH
    total128 = nchunk * CH

    # token -> cache-block source (pad/invalid tokens point at block 0 of
    # the owning seq; uncovered gap rows keep blocks=0 and are never read)
    nb = (ctx + BS - 1) // BS
    blocks = np.zeros(total128, dtype=np.int64)
    offs = np.zeros(total128, dtype=np.int64)
    covered = np.zeros(total128, dtype=bool)
    invalid = np.zeros(total128, dtype=bool)
    for p in range(B):
        s0 = int(starts[p])
        L = int(ltok[p])
        loc = np.arange(L)
        blocks[s0 : s0 + L] = bts[p, np.minimum(loc // BS, int(nb[p]) - 1)]
        offs[s0 : s0 + L] = loc % BS
        covered[s0 : s0 + L] = True
        invalid[s0 + int(ctx[p]) : s0 + L] = True

    # per-chunk exp info: rmax (rows covered by fragments — uncovered rows
    # are always a contiguous chunk tail) + bias column for invalid rows
    bias_cols = []
    chunk_bias = [-1] * nchunk
    chunk_rmax = [CH] * nchunk
    for c in range(nchunk):
        cov = covered[c * CH : c * CH + CH]
        rmax = int(np.max(np.nonzero(cov)[0])) + 1 if cov.any() else 0
        assert cov[:rmax].all(), f"non-tail gap in chunk {c}"
        chunk_rmax[c] = rmax
        inv = invalid[c * CH : c * CH + CH]
        if inv[:rmax].any():
            chunk_bias[c] = len(bias_cols)
            bias_cols.append(np.where(inv, NEG, 0.0).astype(np.float32))
    nbias = max(1, len(bias_cols))
    bias_tab = np.zeros((CH, nbias), dtype=np.float32)
    for k, col in enumerate(bias_cols):
        bias_tab[:, k] = col

    # fragments per chunk: (orig_seq, slot, r0, lp, is_first, is_last) —
    # slot p is the drain-order output position, orig seq id indexes qt
    chunk_frags = [[] for _ in range(nchunk)]
    for p in range(B):
        pieces = []
        t = int(starts[p])
        end = t + int(ltok[p])
        while t < end:
            c = t // CH
            r0 = t % CH
            lp = min(end - t, CH - r0)
            for fr0, flp in _split_frag(r0, lp):
                pieces.append((c, fr0, flp))
            t += lp
        for k, (c, fr0, flp) in enumerate(pieces):
            chunk_frags[c].append(
                (int(order[p]), p, fr0, flp, k == 0, k == len(pieces) - 1)
            )
    assert max(len(f) for f in chunk_frags) <= 3

    # batch sizes in chunks: bulk 32s, then 8-chunk tail batches (fewer
    # serialized DMA->S->exp->O stage chains than a halving taper)
    sizes = []
    rem = nchunk
    while rem > 32:
        sizes.append(BATCH_CHUNKS)
        rem -= BATCH_CHUNKS
    while rem > 0:
        k = min(8, rem)
        sizes.append(k)
        rem -= k

    return {
        "total": total,
        "nchunk": nchunk,
        "batch_sizes": sizes,
        "chunk_frags": chunk_frags,
        "chunk_bias": chunk_bias,
        "chunk_rmax": chunk_rmax,
        "nbias": nbias,
        "bias_tab": bias_tab,
        "blocks": blocks,
        "offs": offs,
        "order": order,
    }


def pack_inputs(plan, q, k_cache, v_cache):
    q = np.asarray(q, dtype=np.float32)
    k_cache = np.asarray(k_cache, dtype=np.float32)
    v_cache = np.asarray(v_cache, dtype=np.float32)

    nchunk = plan["nchunk"]
    blocks, offs = plan["blocks"], plan["offs"]  # [total128], gaps -> block 0

    # one gather for all kv heads: [total128, KVH, D]
    kg = k_cache[blocks, offs]
    vg = v_cache[blocks, offs]

    kv_srcs = []
    for i in range(KVH):
        k3 = kg[:, i, :].reshape(nchunk, CH, D).transpose(0, 2, 1)  # [nc, d, tok]
        v3 = vg[:, i, :].reshape(nchunk, CH, D)  # [nc, tok, d]
        ones3 = np.ones((nchunk, CH, 1), dtype=np.float32)
        kv3 = np.concatenate([k3, v3, ones3], axis=2)  # [nc, 128, 257]
        kv_srcs.append(
            np.ascontiguousarray(kv3.transpose(1, 0, 2).reshape(CH, nchunk * CW)).astype(BF16)
        )

    qs = (q.reshape(B, KVH, G, D) * SCALE).astype(BF16)
    qt = np.ascontiguousarray(qs.transpose(1, 3, 0, 2)).reshape(KVH, D, B * G)
    return kv_srcs, qt


def build(plan):
    """Build the (SPMD-identical) Bacc graph for one core."""
    import concourse.mybir as mybir
    import concourse.tile as tile
    from concourse import bacc

    f32 = mybir.dt.float32
    bf16 = mybir.dt.bfloat16
    EXP = mybir.ActivationFunctionType.Exp

    nchunk = plan["nchunk"]
    batch_sizes = plan["batch_sizes"]
    chunk_frags = plan["chunk_frags"]
    chunk_bias = plan["chunk_bias"]
    chunk_rmax = plan["chunk_rmax"]

    nc = bacc.Bacc()

    kv_ext = nc.declare_dram_parameter("kv", [CH, nchunk * CW], bf16, isOutput=False)
    qt_ext = nc.declare_dram_parameter("qt", [D, B * G], bf16, isOutput=False)
    bias_ext = nc.declare_dram_parameter("bias", [CH, plan["nbias"]], f32, isOutput=False)
    out_ext = nc.declare_dram_parameter("out", [G, B * E], f32, isOutput=True)

    with tile.TileContext(nc) as tc:
        with (
            tc.tile_pool(name="const", bufs=1) as const_pool,
            tc.tile_pool(name="kv", bufs=6) as kv_pool,
            tc.tile_pool(name="pt", bufs=4) as pt_pool,
            tc.tile_pool(name="st_psum", bufs=3, space="PSUM") as st_pool,
            tc.tile_pool(name="o_psum", bufs=5, space="PSUM") as o_pool,
            tc.tile_pool(name="outp", bufs=1) as out_pool,
        ):
            qt_sb = const_pool.tile([D, B * G], bf16, name="qt_sb")
            bias_sb = const_pool.tile([CH, plan["nbias"]], f32, name="bias_sb")
            out_sb = out_pool.tile([G, B * E], f32, name="out_sb")

            o_tiles = {}
            drained = 0
            out_done = 0

            c0 = 0
            for bi, bc in enumerate(batch_sizes):
                kv_t = kv_pool.tile([CH, CW * bc], bf16, tag="kv", name=f"kv{bi}")
                nc.sync.dma_start(
                    out=kv_t[:, :], in_=kv_ext[:, CW * c0 : CW * (c0 + bc)]
                )
                if bi == 0:
                    # constants after the first kv trigger: kv0's transfer
                    # covers their load, and kv0 starts ~1.2us earlier
                    nc.sync.dma_start(out=qt_sb[:, :], in_=qt_ext[:, :])
                    nc.sync.dma_start(out=bias_sb[:, :], in_=bias_ext[:, :])
                g0 = c0
                while g0 < c0 + bc:
                    gc = min(NCHG, c0 + bc - g0)
                    st_t = st_pool.tile([CH, G * gc], f32, tag="st", name=f"st{g0}")
                    pt_t = pt_pool.tile([CH, G * gc], bf16, tag="pt", name=f"pt{g0}")

                    for c in range(g0, g0 + gc):
                        j = c - g0
                        lc = c - c0
                        for s, p, r0, lp, first, last in chunk_frags[c]:
                            # skip_group_check: each S-matmul is an atomic
                            # start+stop single; CoreSim's zero-region check
                            # is bank-granular and false-positives on two
                            # fragments at different partition offsets
                            nc.tensor.matmul(
                                out=st_t[r0 : r0 + lp, G * j : G * (j + 1)],
                                lhsT=kv_t[:, CW * lc + r0 : CW * lc + r0 + lp],
                                rhs=qt_sb[:, G * s : G * (s + 1)],
                                start=True,
                                stop=True,
                                skip_group_check=True,
                                tile_position=(0, r0),
                            )

                    # exps: coalesce runs of plain full chunks; boundary /
                    # stream-end chunks get their own (bias / short) exp
                    run = None
                    for c in range(g0, g0 + gc + 1):
                        plain = (
                            c < g0 + gc
                            and chunk_bias[c] < 0
                            and chunk_rmax[c] == CH
                        )
                        if plain:
                            run = c if run is None else run
                            continue
                        if run is not None:
                            a, b = run - g0, c - g0
                            nc.scalar.activation(
                                pt_t[:, G * a : G * b], st_t[:, G * a : G * b], EXP
                            )
                            run = None
                        if c < g0 + gc:
                            j = c - g0
                            rm = chunk_rmax[c]
                            bk = chunk_bias[c]
                            if rm == 0:
                                pass  # fully uncovered chunk: nothing to exp
                            elif bk >= 0:
                                nc.scalar.activation(
                                    pt_t[0:rm, G * j : G * (j + 1)],
                                    st_t[0:rm, G * j : G * (j + 1)],
                                    EXP,
                                    bias=bias_sb[0:rm, bk : bk + 1],
                                )
                            else:
                                nc.scalar.activation(
                                    pt_t[0:rm, G * j : G * (j + 1)],
                                    st_t[0:rm, G * j : G * (j + 1)],
                                    EXP,
                                )

                    for c in range(g0, g0 + gc):
                        j = c - g0
                        lc = c - c0
                        for s, p, r0, lp, first, last in chunk_frags[c]:
                            if first:
                                o_tiles[p] = o_pool.tile(
                                    [G, E], f32, tag="o", name=f"o{p}"
                                )
                            nc.tensor.matmul(
                                out=o_tiles[p][:, :],
                                lhsT=pt_t[r0 : r0 + lp, G * j : G * (j + 1)],
                                rhs=kv_t[r0 : r0 + lp, CW * lc + D : CW * lc + CW],
                                start=first,
                                stop=last,
                                tile_position=(r0, 0),
                            )
                            if last:
                                # drain slot p: drain order == slot order
                                nc.vector.tensor_copy(
                                    out=out_sb[:, E * p : E * (p + 1)],
                                    in_=o_tiles[p][:, :],
                                )
                                del o_tiles[p]
                                drained += 1
                                if drained in (24, 48):
                                    nc.sync.dma_start(
                                        out=out_ext[:, E * out_done : E * drained],
                                        in_=out_sb[:, E * out_done : E * drained],
                                    )
                                    out_done = drained
                    g0 += gc
                c0 += bc

            nc.sync.dma_start(
                out=out_ext[:, E * out_done :], in_=out_sb[:, E * out_done :]
            )

    nc.compile()
    return nc


def _assemble(results, order):
    inv = np.argsort(order)  # orig seq -> drain slot
    outs = []
    for i in range(NCORES):
        o = np.asarray(results[i]["out"], dtype=np.float32).reshape(G, B, E)
        o = o[:, inv, :]  # un-permute drain slots back to seq order
        outs.append(o[:, :, :D] / o[:, :, D : D + 1])
    # [KVH, G, B, D] -> [B, KVH, G, D] -> [B, H, D]
    return (
        np.stack(outs, axis=0).transpose(2, 0, 1, 3).reshape(B, H, D).astype(np.float32)
    )


def kernel(q, k_cache, v_cache, block_tables, context_lens, _trace=False):
    from concourse.bass_utils import run_bass_kernel_spmd

    plan = plan_problem(block_tables, context_lens)
    kv_srcs, qt = pack_inputs(plan, q, k_cache, v_cache)
    nc = build(plan)
    in_maps = [
        {"kv": kv_srcs[i], "qt": qt[i], "bias": plan["bias_tab"]}
        for i in range(NCORES)
    ]
    res = run_bass_kernel_spmd(nc, in_maps, core_ids=list(range(NCORES)), trace=_trace)
    out = _assemble(res.results, plan["order"])
    if _trace:
        return out, res
    return out



# revision 6
# speedup vs baseline: 1.1944x; 1.1944x over previous
"""Decode-phase paged attention with GQA on 8 TRN2 NeuronCores — v2.

Sharding: kv-head axis across the 8 cores (1 kv head + its 4 query heads per
core); q/block_tables/context_lens replicated, metadata baked into the
per-call compiled graph (host resolves the paged gather, device streams every
gathered KV byte from HBM — the memory-bound roofline term).

v2 layout vs v1:
- Long sequences are packed CONTINUOUSLY at 32-token granularity (v1 padded
  every sequence to 128): fewer HBM bytes. Sequences start mid-chunk;
  matmuls address partition offsets via tile_position (fragments split so
  (offset, len) hits legal 32-aligned placements).
- K, V and a ones-column are interleaved per 128-token chunk in ONE dram
  tensor ([d,128]K | [128,128]V | ones), so each batch is a single ~2 MB DMA
  on the sync queue — no second queue serialized behind compute.
- exp() is emitted per 16-chunk group, coalescing full chunks into one ACT
  instruction; boundary chunks get a host-built -60 bias column (masks both
  ctx tail and 32-pad rows).
- The softmax divide moved to the host: the device ships numerator and
  denominator ([G, 129] per seq, PSUM->SBUF drained on DVE), which takes the
  reciprocal+normalize chain off the critical tail and keeps ACT exp-only.
- Batch sizes taper (32,... then 8-chunk tail batches) so the compute tail
  after the last DMA byte is tiny.
- Sequences are packed shortest-first (the tail after the last DMA is one
  long sequence's final chunks), and a chunk never holds more than 3 matmul
  fragments — 4-fragment chunks (four 32-row tile_position'd matmuls) hard-
  fault the device; starts bump to the next chunk boundary when needed.
"""

import sys

if "/opt/trn_rl_repo" not in sys.path:
    sys.path.insert(0, "/opt/trn_rl_repo")

import numpy as np
import ml_dtypes

B = 64
H = 32
KVH = 8
G = H // KVH  # 4
D = 128
BS = 16  # tokens per cache block
NB = 8192  # blocks in cache
CH = 128  # tokens per chunk (compute tile)
CW = 2 * D + 1  # kv stream cols per chunk: K(128) | V(128) | ones(1)
E = D + 1  # per-seq output cols: numerator(128) | denominator(1)
ALIGN = 32  # long-seq token padding granularity
SHORT_MAX = 384  # ctx at/below this pads to full chunks (fragment-run guard)
NCHG = 16  # chunks per st/pt group (one exp batch)
BATCH_CHUNKS = 32  # chunks per KV buffer
SCALE = 0.08838834764831845
NCORES = 8
NEG = -60.0

BF16 = ml_dtypes.bfloat16
FP8 = ml_dtypes.float8_e3m4  # kv stream dtype: 4 mantissa bits, range +-15.5


def _split_frag(r0, lp):
    """Split a (row0, len) piece so each part is a legal tile placement.

    Legal (pos, size): pos 0 any size; pos 32 size<=32; pos 64 size<=64;
    pos 96 size<=32 (round-up sizes 32/64/128 vs 32-aligned positions).
    """
    if r0 == 32 and lp > 32:
        return [(32, 32), (64, lp - 32)]
    return [(r0, lp)]


def plan_problem(block_tables, context_lens):
    bt = np.asarray(block_tables, dtype=np.int64)
    ctx_orig = np.asarray(context_lens, dtype=np.int64)

    # pack shortest-first so the post-last-DMA tail is one long sequence's
    # final chunks -> shortest possible compute/drain chain at the end
    order = np.argsort(ctx_orig, kind="stable")
    ctx = ctx_orig[order]
    bts = bt[order]

    # Short seqs pad to full 128-token chunks; long seqs pad to 32. Runs of
    # 3+ consecutive fragment-dense chunks hard-fault the device, so short
    # seqs (which would pack several 32-row fragments per chunk back-to-
    # back) get whole chunks to themselves, and long-seq boundary chunks
    # are always separated by >=2 full interior chunks. Sorted ascending,
    # the shorts come first with 128-aligned starts for free.
    short = ctx <= SHORT_MAX
    ltok = np.where(
        short,
        np.maximum(CH, ((ctx + CH - 1) // CH) * CH),
        ((ctx + ALIGN - 1) // ALIGN) * ALIGN,
    )

    # sequential packing with a fragment cap: a chunk must never hold 4+
    # matmul fragments (dense 32-row fragment runs hard-fault the device).
    # If starting a seq mid-chunk would push that chunk past 3 fragments,
    # bump the seq to the next chunk boundary (the skipped rows stay
    # uncovered: no fragment touches them and exps exclude them via rmax).
    starts = np.zeros(B, dtype=np.int64)
    frag_cnt = {}
    cur = 0
    for p in range(B):
        L = int(ltok[p])
        r = cur % CH
        if r:
            head = min(L, CH - r)
            if frag_cnt.get(cur // CH, 0) + len(_split_frag(r, head)) > 3:
                cur = (cur // CH + 1) * CH
        starts[p] = cur
        t = cur
        end = cur + L
        while t < end:
            c = t // CH
            r0 = t % CH
            lp = min(end - t, CH - r0)
            frag_cnt[c] = frag_cnt.get(c, 0) + len(_split_frag(r0, lp))
            t += lp
        cur = end
    total = cur
    nchunk = (total + CH - 1) // CH
    total128 = nchunk * CH

    # token -> cache-block source (pad/invalid tokens point at block 0 of
    # the owning seq; uncovered gap rows keep blocks=0 and are never read)
    nb = (ctx + BS - 1) // BS
    blocks = np.zeros(total128, dtype=np.int64)
    offs = np.zeros(total128, dtype=np.int64)
    covered = np.zeros(total128, dtype=bool)
    invalid = np.zeros(total128, dtype=bool)
    for p in range(B):
        s0 = int(starts[p])
        L = int(ltok[p])
        loc = np.arange(L)
        blocks[s0 : s0 + L] = bts[p, np.minimum(loc // BS, int(nb[p]) - 1)]
        offs[s0 : s0 + L] = loc % BS
        covered[s0 : s0 + L] = True
        invalid[s0 + int(ctx[p]) : s0 + L] = True

    # per-chunk exp info: rmax (rows covered by fragments — uncovered rows
    # are always a contiguous chunk tail) + bias column for invalid rows
    bias_cols = []
    chunk_bias = [-1] * nchunk
    chunk_rmax = [CH] * nchunk
    for c in range(nchunk):
        cov = covered[c * CH : c * CH + CH]
        rmax = int(np.max(np.nonzero(cov)[0])) + 1 if cov.any() else 0
        assert cov[:rmax].all(), f"non-tail gap in chunk {c}"
        chunk_rmax[c] = rmax
        inv = invalid[c * CH : c * CH + CH]
        if inv[:rmax].any():
            chunk_bias[c] = len(bias_cols)
            bias_cols.append(np.where(inv, NEG, 0.0).astype(np.float32))
    nbias = max(1, len(bias_cols))
    bias_tab = np.zeros((CH, nbias), dtype=np.float32)
    for k, col in enumerate(bias_cols):
        bias_tab[:, k] = col

    # fragments per chunk: (orig_seq, slot, r0, lp, is_first, is_last) —
    # slot p is the drain-order output position, orig seq id indexes qt
    chunk_frags = [[] for _ in range(nchunk)]
    for p in range(B):
        pieces = []
        t = int(starts[p])
        end = t + int(ltok[p])
        while t < end:
            c = t // CH
            r0 = t % CH
            lp = min(end - t, CH - r0)
            for fr0, flp in _split_frag(r0, lp):
                pieces.append((c, fr0, flp))
            t += lp
        for k, (c, fr0, flp) in enumerate(pieces):
            chunk_frags[c].append(
                (int(order[p]), p, fr0, flp, k == 0, k == len(pieces) - 1)
            )
    assert max(len(f) for f in chunk_frags) <= 3

    # batch sizes in chunks: bulk 32s, then 8-chunk tail batches (fewer
    # serialized DMA->S->exp->O stage chains than a halving taper)
    sizes = []
    rem = nchunk
    while rem > 32:
        sizes.append(BATCH_CHUNKS)
        rem -= BATCH_CHUNKS
    while rem > 0:
        k = min(8, rem)
        sizes.append(k)
        rem -= k

    return {
        "total": total,
        "nchunk": nchunk,
        "batch_sizes": sizes,
        "chunk_frags": chunk_frags,
        "chunk_bias": chunk_bias,
        "chunk_rmax": chunk_rmax,
        "nbias": nbias,
        "bias_tab": bias_tab,
        "blocks": blocks,
        "offs": offs,
        "order": order,
    }


def pack_inputs(plan, q, k_cache, v_cache):
    q = np.asarray(q, dtype=np.float32)
    k_cache = np.asarray(k_cache, dtype=np.float32)
    v_cache = np.asarray(v_cache, dtype=np.float32)

    nchunk = plan["nchunk"]
    blocks, offs = plan["blocks"], plan["offs"]  # [total128], gaps -> block 0

    # one gather for all kv heads: [total128, KVH, D]
    kg = k_cache[blocks, offs]
    vg = v_cache[blocks, offs]

    kv_srcs = []
    for i in range(KVH):
        k3 = kg[:, i, :].reshape(nchunk, CH, D).transpose(0, 2, 1)  # [nc, d, tok]
        v3 = vg[:, i, :].reshape(nchunk, CH, D)  # [nc, tok, d]
        ones3 = np.ones((nchunk, CH, 1), dtype=np.float32)
        kv3 = np.concatenate([k3, v3, ones3], axis=2)  # [nc, 128, 257]
        kv_srcs.append(
            np.ascontiguousarray(kv3.transpose(1, 0, 2).reshape(CH, nchunk * CW)).astype(FP8)
        )

    qs = (q.reshape(B, KVH, G, D) * SCALE).astype(BF16)
    qt = np.ascontiguousarray(qs.transpose(1, 3, 0, 2)).reshape(KVH, D, B * G)
    return kv_srcs, qt


def build(plan):
    """Build the (SPMD-identical) Bacc graph for one core."""
    import concourse.mybir as mybir
    import concourse.tile as tile
    from concourse import bacc

    f32 = mybir.dt.float32
    bf16 = mybir.dt.bfloat16
    fp8 = mybir.dt.float8e3
    EXP = mybir.ActivationFunctionType.Exp

    nchunk = plan["nchunk"]
    batch_sizes = plan["batch_sizes"]
    chunk_frags = plan["chunk_frags"]
    chunk_bias = plan["chunk_bias"]
    chunk_rmax = plan["chunk_rmax"]

    nc = bacc.Bacc()

    kv_ext = nc.declare_dram_parameter("kv", [CH, nchunk * CW], fp8, isOutput=False)
    qt_ext = nc.declare_dram_parameter("qt", [D, B * G], bf16, isOutput=False)
    bias_ext = nc.declare_dram_parameter("bias", [CH, plan["nbias"]], f32, isOutput=False)
    out_ext = nc.declare_dram_parameter("out", [G, B * E], f32, isOutput=True)

    with tile.TileContext(nc) as tc:
        with (
            tc.tile_pool(name="const", bufs=1) as const_pool,
            tc.tile_pool(name="kv", bufs=6) as kv_pool,
            tc.tile_pool(name="pt", bufs=4) as pt_pool,
            tc.tile_pool(name="st_psum", bufs=3, space="PSUM") as st_pool,
            tc.tile_pool(name="o_psum", bufs=5, space="PSUM") as o_pool,
            tc.tile_pool(name="outp", bufs=1) as out_pool,
        ):
            qt_sb = const_pool.tile([D, B * G], bf16, name="qt_sb")
            bias_sb = const_pool.tile([CH, plan["nbias"]], f32, name="bias_sb")
            out_sb = out_pool.tile([G, B * E], f32, name="out_sb")

            o_tiles = {}
            drained = 0
            out_done = 0

            c0 = 0
            for bi, bc in enumerate(batch_sizes):
                kv_t = kv_pool.tile([CH, CW * bc], fp8, tag="kv", name=f"kv{bi}")
                nc.sync.dma_start(
                    out=kv_t[:, :], in_=kv_ext[:, CW * c0 : CW * (c0 + bc)]
                )
                if bi == 0:
                    # constants after the first kv trigger: kv0's transfer
                    # covers their load, and kv0 starts ~1.2us earlier
                    nc.sync.dma_start(out=qt_sb[:, :], in_=qt_ext[:, :])
                    nc.sync.dma_start(out=bias_sb[:, :], in_=bias_ext[:, :])
                g0 = c0
                while g0 < c0 + bc:
                    gc = min(NCHG, c0 + bc - g0)
                    st_t = st_pool.tile([CH, G * gc], f32, tag="st", name=f"st{g0}")
                    pt_t = pt_pool.tile([CH, G * gc], bf16, tag="pt", name=f"pt{g0}")

                    for c in range(g0, g0 + gc):
                        j = c - g0
                        lc = c - c0
                        for s, p, r0, lp, first, last in chunk_frags[c]:
                            # skip_group_check: each S-matmul is an atomic
                            # start+stop single; CoreSim's zero-region check
                            # is bank-granular and false-positives on two
                            # fragments at different partition offsets
                            nc.tensor.matmul(
                                out=st_t[r0 : r0 + lp, G * j : G * (j + 1)],
                                lhsT=kv_t[:, CW * lc + r0 : CW * lc + r0 + lp],
                                rhs=qt_sb[:, G * s : G * (s + 1)],
                                start=True,
                                stop=True,
                                skip_group_check=True,
                                tile_position=(0, r0),
                            )

                    # exps: coalesce runs of plain full chunks; boundary /
                    # stream-end chunks get their own (bias / short) exp
                    run = None
                    for c in range(g0, g0 + gc + 1):
                        plain = (
                            c < g0 + gc
                            and chunk_bias[c] < 0
                            and chunk_rmax[c] == CH
                        )
                        if plain:
                            run = c if run is None else run
                            continue
                        if run is not None:
                            a, b = run - g0, c - g0
                            nc.scalar.activation(
                                pt_t[:, G * a : G * b], st_t[:, G * a : G * b], EXP
                            )
                            run = None
                        if c < g0 + gc:
                            j = c - g0
                            rm = chunk_rmax[c]
                            bk = chunk_bias[c]
                            if rm == 0:
                                pass  # fully uncovered chunk: nothing to exp
                            elif bk >= 0:
                                nc.scalar.activation(
                                    pt_t[0:rm, G * j : G * (j + 1)],
                                    st_t[0:rm, G * j : G * (j + 1)],
                                    EXP,
                                    bias=bias_sb[0:rm, bk : bk + 1],
                                )
                            else:
                                nc.scalar.activation(
                                    pt_t[0:rm, G * j : G * (j + 1)],
                                    st_t[0:rm, G * j : G * (j + 1)],
                                    EXP,
                                )

                    for c in range(g0, g0 + gc):
                        j = c - g0
                        lc = c - c0
                        for s, p, r0, lp, first, last in chunk_frags[c]:
                            if first:
                                o_tiles[p] = o_pool.tile(
                                    [G, E], f32, tag="o", name=f"o{p}"
                                )
                            nc.tensor.matmul(
                                out=o_tiles[p][:, :],
                                lhsT=pt_t[r0 : r0 + lp, G * j : G * (j + 1)],
                                rhs=kv_t[r0 : r0 + lp, CW * lc + D : CW * lc + CW],
                                start=first,
                                stop=last,
                                tile_position=(r0, 0),
                            )
                            if last:
                                # drain slot p: drain order == slot order
                                nc.vector.tensor_copy(
                                    out=out_sb[:, E * p : E * (p + 1)],
                                    in_=o_tiles[p][:, :],
                                )
                                del o_tiles[p]
                                drained += 1
                                if drained in (24, 48):
                                    nc.sync.dma_start(
                                        out=out_ext[:, E * out_done : E * drained],
                                        in_=out_sb[:, E * out_done : E * drained],
                                    )
                                    out_done = drained
                    g0 += gc
                c0 += bc

            nc.sync.dma_start(
                out=out_ext[:, E * out_done :], in_=out_sb[:, E * out_done :]
            )

    nc.compile()
    return nc


def _assemble(results, order):
    inv = np.argsort(order)  # orig seq -> drain slot
    outs = []
    for i in range(NCORES):
        o = np.asarray(results[i]["out"], dtype=np.float32).reshape(G, B, E)
        o = o[:, inv, :]  # un-permute drain slots back to seq order
        outs.append(o[:, :, :D] / o[:, :, D : D + 1])
    # [KVH, G, B, D] -> [B, KVH, G, D] -> [B, H, D]
    return (
        np.stack(outs, axis=0).transpose(2, 0, 1, 3).reshape(B, H, D).astype(np.float32)
    )


def kernel(q, k_cache, v_cache, block_tables, context_lens, _trace=False):
    from concourse.bass_utils import run_bass_kernel_spmd

    plan = plan_problem(block_tables, context_lens)
    kv_srcs, qt = pack_inputs(plan, q, k_cache, v_cache)
    nc = build(plan)
    in_maps = [
        {"kv": kv_srcs[i], "qt": qt[i], "bias": plan["bias_tab"]}
        for i in range(NCORES)
    ]
    res = run_bass_kernel_spmd(nc, in_maps, core_ids=list(range(NCORES)), trace=_trace)
    out = _assemble(res.results, plan["order"])
    if _trace:
        return out, res
    return out

